# revision 11
# baseline (speedup 1.0000x reference)
"""Trainium2 Bass kernel for nn_CAN_Layer_74775380623980.

Math: with sequence length L=1, softmax over the single key is exactly 1.0
and the reference's masks are overwritten with ones, so the whole cross
attention collapses to

    E   = (protein @ Wv_p + drug @ Wv_d) / 2          # [N, 2048]
    out = concat([E, E], axis=1)                      # [N, 4096]

Sharding: pure data parallel, batch N=16384 split 8 ways (2048 rows/core);
the two V projection weights are replicated. Per core the device computes
E_shard = Xp @ (Wv_p/2) + Xd @ (Wv_d/2) as fp16 matmuls (fp32 PSUM
accumulation). The 0.5 scale is folded into the weights on the host (exact,
power of two). Activations are pre-transposed/tiled on the host so every DMA
is contiguous 4KB-per-partition and the PE runs K-contiguous back-to-back
matmuls with no on-device transposes.
"""

import numpy as np

P = 128          # partitions / systolic tile
N_FULL = 16384
D = 2048         # contraction dim
HID = 2048       # output dim per projection
NCORES = 8
M_SH = N_FULL // NCORES   # 2048 rows per core
KT = D // P               # 16 k-tiles
NBLK = 512                # matmul free dim (one PSUM bank of fp32)
NB = HID // NBLK          # 4 n-blocks
MT_FULL = M_SH // P       # 16 m-tiles


def _build_module(mt_tiles=MT_FULL, reps=1):
    """reps>1 wraps the m-loop in a device-side For_i — used only for
    wall-clock benchmarking (amplifies device time above RPC noise)."""
    import concourse.bass as bass  # noqa: F401
    import concourse.mybir as mybir
    import concourse.tile as tile
    from concourse import bacc

    fp16 = mybir.dt.float16
    f32 = mybir.dt.float32

    nc = bacc.Bacc("TRN2", target_bir_lowering=False, debug=False)

    xp_h = nc.dram_tensor("xp", [mt_tiles, P, KT, P], fp16, kind="ExternalInput")
    xd_h = nc.dram_tensor("xd", [mt_tiles, P, KT, P], fp16, kind="ExternalInput")
    wp_h = nc.dram_tensor("wp", [KT, P, HID], fp16, kind="ExternalInput")
    wd_h = nc.dram_tensor("wd", [KT, P, HID], fp16, kind="ExternalInput")
    out_h = nc.dram_tensor("out", [mt_tiles * P, HID], f32, kind="ExternalOutput")

    with tile.TileContext(nc) as tc:
        with (
            tc.tile_pool(name="wpool", bufs=1) as wpool,
            tc.tile_pool(name="xpool", bufs=2) as xpool,
            tc.tile_pool(name="opool", bufs=2) as opool,
            tc.tile_pool(name="psum", bufs=2, space="PSUM") as pp,
        ):
            x_tiles = {}

            def load_x(mt):
                tp = xpool.tile([P, KT, P], fp16, tag="xp", name=f"xp_{mt}")
                nc.sync.dma_start(tp[:], xp_h[mt])
                td = xpool.tile([P, KT, P], fp16, tag="xd", name=f"xd_{mt}")
                nc.sync.dma_start(td[:], xd_h[mt])
                x_tiles[mt] = (tp, td)

            wp_sb, wd_sb = [], []

            def load_weights():
                wp_sb.clear()
                wd_sb.clear()
                for j in range(KT):
                    tw = wpool.tile([P, HID], fp16, tag=f"wp{j}", name=f"wp_{j}")
                    nc.sync.dma_start(tw[:], wp_h[j])
                    wp_sb.append(tw)
                    tw = wpool.tile([P, HID], fp16, tag=f"wd{j}", name=f"wd_{j}")
                    nc.sync.dma_start(tw[:], wd_h[j])
                    wd_sb.append(tw)

            def m_loop():
                for mt in range(mt_tiles):
                    if mt + 1 < mt_tiles:
                        load_x(mt + 1)
                    xp_t, xd_t = x_tiles.pop(mt)
                    psums = [
                        pp.tile([P, NBLK], f32, tag=f"ps{nb}", name=f"ps_{mt}_{nb}")
                        for nb in range(NB)
                    ]
                    for j in range(KT):
                        for nb in range(NB):
                            nc.tensor.matmul(
                                psums[nb][:],
                                xp_t[:, j, :],
                                wp_sb[j][:, nb * NBLK : (nb + 1) * NBLK],
                                start=(j == 0),
                                stop=False,
                            )
                        for nb in range(NB):
                            nc.tensor.matmul(
                                psums[nb][:],
                                xd_t[:, j, :],
                                wd_sb[j][:, nb * NBLK : (nb + 1) * NBLK],
                                start=False,
                                stop=(j == KT - 1),
                            )
                    out_t = opool.tile([P, HID], f32, tag="out", name=f"out_{mt}")
                    for nb in range(NB):
                        nc.vector.tensor_copy(
                            out_t[:, nb * NBLK : (nb + 1) * NBLK], psums[nb][:]
                        )
                    nc.sync.dma_start(out_h[mt * P : (mt + 1) * P, :], out_t[:])

            if reps == 1:
                # first m-tile's activations go out before the weight preload
                # so the PE starts as soon as strip j=0 of the weights lands
                load_x(0)
                load_weights()
                m_loop()
            else:
                # full body (weight preload included) repeats: per-rep wall
                # time == one-shot kernel exec time
                with tc.For_i(0, reps, 1):
                    load_x(0)
                    load_weights()
                    m_loop()

    nc.compile()
    return nc


def _prep_inputs(protein, drug, Wv_p, Wv_d, mt_tiles=MT_FULL):
    """Host-side shard + transpose-tile + fp16 cast."""
    wp = (0.5 * np.asarray(Wv_p, dtype=np.float32)).astype(np.float16)
    wd = (0.5 * np.asarray(Wv_d, dtype=np.float32)).astype(np.float16)
    wp = np.ascontiguousarray(wp.reshape(KT, P, HID))
    wd = np.ascontiguousarray(wd.reshape(KT, P, HID))

    def tile_x(x):
        # [M_SH, D] -> [mt, p, j, m'] with x_t[mt, p, j, m'] = x[mt*P+m', j*P+p]
        t = x.reshape(mt_tiles, P, KT, P).transpose(0, 3, 2, 1)
        return np.ascontiguousarray(t.astype(np.float16))

    protein = np.asarray(protein, dtype=np.float32)
    drug = np.asarray(drug, dtype=np.float32)
    in_maps = []
    rows = mt_tiles * P
    for c in range(NCORES):
        sl = slice(c * M_SH, c * M_SH + rows)
        in_maps.append(
            {
                "xp": tile_x(protein[sl]),
                "xd": tile_x(drug[sl]),
                "wp": wp,
                "wd": wd,
            }
        )
    return in_maps


_MODULE_CACHE = {}


def _run(protein, drug, Wv_p, Wv_d, trace=False, mt_tiles=MT_FULL):
    from concourse.bass_utils import run_bass_kernel_spmd

    nc = _MODULE_CACHE.get(mt_tiles)
    if nc is None:
        nc = _MODULE_CACHE[mt_tiles] = _build_module(mt_tiles)
    in_maps = _prep_inputs(protein, drug, Wv_p, Wv_d, mt_tiles)
    res = run_bass_kernel_spmd(nc, in_maps, list(range(NCORES)), trace=trace)
    E = np.concatenate(
        [np.asarray(r["out"], dtype=np.float32) for r in res.results], axis=0
    )
    return E, res


def kernel(
    protein,
    drug,
    mask_prot=None,
    mask_drug=None,
    Wq_p=None,
    Wk_p=None,
    Wv_p=None,
    Wq_d=None,
    Wk_d=None,
    Wv_d=None,
):
    E, _ = _run(protein, drug, Wv_p, Wv_d, trace=False)
    return np.concatenate([E, E], axis=1)


def kernel_profiled(**inputs):
    E, res = _run(
        inputs["protein"], inputs["drug"], inputs["Wv_p"], inputs["Wv_d"], trace=False
    )
    out = np.concatenate([E, E], axis=1)
    return out, res
